# revision 9
# baseline (speedup 1.0000x reference)
"""Binarized 3x3 conv (BConv) Trainium2 Bass kernel — Winograd F(2x2,3x3).

Problem: x[32,256,56,56] f32, W[256,256,3,3] f32.
  out = conv2d(x, sign(W), stride 1, pad 1)  (NCHW / OIHW)

Strategy:
  - Data-parallel over batch: 8 cores x 4 images each, identical SPMD program.
  - Winograd F(2x2,3x3): 2.25x fewer MACs than direct conv. Output tiles are
    2x2; 28x28 tiles per image. The output *column* transform A is folded
    into the weights (P[u,j] = sum_v a_jv M[u,v] becomes extra matmul
    accumulation steps with pre-signed weight copies), so PSUM holds 8
    P-tiles per chunk instead of 16 M-tiles and the whole column-stage of
    the output transform disappears from the vector engines.
  - Per (img, oc-half, 14-tile-row chunk): 8 PSUM tiles P[u,j] of
    [128 oc, 14t, 28s] (N=392), each accumulating 6 matmuls (3 v-taps x 2
    ic-halves), bf16. 384 matmuls of N=392 per core total.
  - Input transform: width-stage on GpSimd (f32 staged image -> bf16 CT,
    column padding handled by interior ops + tiny edge slivers on ACT),
    height-stage on DVE (bf16, unit-stride, 2x mode).
  - Output row-stage (A^T on rows) on DVE, straight out of PSUM into a
    strided SBUF view; contiguous-per-partition DMAs both directions.
  - Weight pipeline on-device: binarize ((w>=0)-0.5 = sign/2), G-transform
    with doubled row coefficients (folds the x2 back in; all values exact
    in bf16), signed copies for the j=1 A-fold.
"""

import sys
from contextlib import ExitStack

sys.path.insert(0, "/opt/trn_rl_repo")

import numpy as np

import concourse.mybir as mybir
import concourse.tile as tile
from concourse import bacc
from concourse.bass_utils import run_bass_kernel_spmd

N_CORES = 8
NIMG = 4          # images per core (32 / 8)
C = 256           # channels (in == out)
H = 56
P = 128           # partitions
T = 28            # winograd tile rows/cols per image
TC = 14           # tile rows per chunk
NCH = T // TC     # 2 chunks per image
NMM = TC * 28     # moving free dim per matmul = 392

F32 = mybir.dt.float32
BF16 = mybir.dt.bfloat16
ADD = mybir.AluOpType.add
SUB = mybir.AluOpType.subtract
MUL = mybir.AluOpType.mult

_cached = {}


def build_program(num_devices=N_CORES):
    nc = bacc.Bacc("TRN2", target_bir_lowering=False, debug=False,
                   num_devices=num_devices)

    x_d = nc.dram_tensor("x", [NIMG, C, H, H], F32, kind="ExternalInput")
    # W host-permuted to [C_in, kh*3+kw, C_out] (layout only, no arithmetic)
    w_d = nc.dram_tensor("W", [C, 9, C], F32, kind="ExternalInput")
    y_d = nc.dram_tensor("y", [NIMG, C, H, H], F32, kind="ExternalOutput")

    with tile.TileContext(nc) as tc, ExitStack() as ctx:
        wst_pool = ctx.enter_context(tc.tile_pool(name="wst", bufs=2))
        wgt_pool = ctx.enter_context(tc.tile_pool(name="wgt", bufs=1))
        stg_pool = ctx.enter_context(tc.tile_pool(name="stg", bufs=3))
        ct_pool = ctx.enter_context(tc.tile_pool(name="ct", bufs=2))
        u_pool = ctx.enter_context(tc.tile_pool(name="u", bufs=4))
        osb_pool = ctx.enter_context(tc.tile_pool(name="osb", bufs=3))
        tmp_pool = ctx.enter_context(tc.tile_pool(name="tmp", bufs=4))
        psum_pool = ctx.enter_context(tc.tile_pool(name="ps", bufs=8,
                                                   space="PSUM"))

        # ---- weight pipeline -------------------------------------------
        # per ic-half: ws = (w>=0)-0.5 (= sign/2, bf16-exact), then the
        # Winograd G-transform with row coefficients doubled (2G), making
        # the net weights G*sign(W)*G^T exactly; all values are multiples
        # of 0.25 with magnitude <= 3 -> exact in bf16.
        wGT, wV1, wV2, wV2n, wGT2n = [], [], [], [], []

        def prep_weights(ic, eng):
            wst = wst_pool.tile([P, 9, C], F32, tag="wst", name=f"wst{ic}")
            dma = nc.sync if ic == 0 else nc.scalar
            dma.dma_start(wst[:], w_d[ic * P:(ic + 1) * P])
            ws = wgt_pool.tile([P, 9, C], BF16, tag=f"ws{ic}", name=f"ws{ic}")
            eng.tensor_scalar(ws[:], wst[:], 0.0, 0.5,
                              mybir.AluOpType.is_ge, SUB)
            gt = wgt_pool.tile([P, 4, 3, C], BF16, tag=f"gt{ic}",
                               name=f"gt{ic}")
            s0, s1, s2 = ws[:, 0:3], ws[:, 3:6], ws[:, 6:9]
            eng.tensor_scalar_mul(gt[:, 0], s0, 2.0)
            eng.tensor_tensor(gt[:, 1], s0, s1, ADD)
            eng.tensor_tensor(gt[:, 1], gt[:, 1], s2, ADD)
            eng.tensor_tensor(gt[:, 2], s0, s1, SUB)
            eng.tensor_tensor(gt[:, 2], gt[:, 2], s2, ADD)
            eng.tensor_scalar_mul(gt[:, 3], s2, 2.0)
            t0, t1, t2 = gt[:, :, 0], gt[:, :, 1], gt[:, :, 2]
            t2h = wgt_pool.tile([P, 4, C], BF16, tag=f"t2h{ic}",
                                name=f"t2h{ic}")
            eng.tensor_scalar_mul(t2h[:], t2, 0.5)
            v1 = wgt_pool.tile([P, 4, C], BF16, tag=f"v1_{ic}",
                               name=f"v1_{ic}")
            q = wgt_pool.tile([P, 4, C], BF16, tag=f"q{ic}", name=f"q{ic}")
            eng.tensor_tensor(q[:], t0, t1, ADD)
            eng.scalar_tensor_tensor(v1[:], q[:], 0.5, t2h[:], MUL, ADD)
            v2 = wgt_pool.tile([P, 4, C], BF16, tag=f"v2_{ic}",
                               name=f"v2_{ic}")
            eng.tensor_tensor(q[:], t0, t1, SUB)
            eng.scalar_tensor_tensor(v2[:], q[:], 0.5, t2h[:], MUL, ADD)
            v2n = wgt_pool.tile([P, 4, C], BF16, tag=f"v2n{ic}",
                                name=f"v2n{ic}")
            eng.tensor_scalar_mul(v2n[:], v2[:], -1.0)
            gt2n = wgt_pool.tile([P, 4, C], BF16, tag=f"gt2n{ic}",
                                 name=f"gt2n{ic}")
            eng.tensor_scalar_mul(gt2n[:], t2, -1.0)
            wGT.append(gt)
            wV1.append(v1)
            wV2.append(v2)
            wV2n.append(v2n)
            wGT2n.append(gt2n)

        # lhsT view for accumulation step (u, j, vslot, ic, oc)
        def wview(u, j, vs, ic, oc):
            ocs = slice(oc * P, (oc + 1) * P)
            if j == 0:
                if vs == 0:
                    return 0, wGT[ic][:, u, 0, ocs]
                if vs == 1:
                    return 1, wV1[ic][:, u, ocs]
                return 2, wV2[ic][:, u, ocs]
            else:
                if vs == 0:
                    return 1, wV1[ic][:, u, ocs]
                if vs == 1:
                    return 2, wV2n[ic][:, u, ocs]
                return 3, wGT2n[ic][:, u, ocs]

        # ---- input pipeline --------------------------------------------
        # stg: rows padded ([58] incl. zero rows 0,57), cols unpadded ->
        # the x DMA lands as one fully-contiguous block per partition.
        # s1 (width/B^T on cols, f32->bf16, GpSimd + ACT slivers):
        #   CT[., r, v, s], v-taps over unpadded cols u = pad-1:
        #     v0 = u[2s-1]-u[2s+1]  (s=0 sliver: -u[1])
        #     v1 = u[2s]+u[2s+1]
        #     v2 = u[2s+1]-u[2s]
        #     v3 = u[2s]-u[2s+2]    (s=27 sliver: u[54])
        # s2 (height/B^T on rows, bf16 2x, DVE):
        #     U[u0] = CT[2t]-CT[2t+2];  U[u1] = CT[2t+1]+CT[2t+2]
        #     U[u2] = CT[2t+2]-CT[2t+1];U[u3] = CT[2t+1]-CT[2t+3]
        def load_image(img):
            stgs, cts = [], []
            for ic in range(2):
                stg = stg_pool.tile([P, H + 2, H], F32, tag="stg",
                                    name=f"stg_{img}_{ic}")
                nc.scalar.memzero(stg[:, 0, :])
                nc.scalar.memzero(stg[:, H + 1, :])
                # two row-bands so chunk-0 transforms unblock early
                for b0, b1 in ((0, 29), (29, H)):
                    dma = nc.scalar if ic == 0 else nc.sync
                    dma.dma_start(stg[:, 1 + b0:1 + b1, :],
                                  x_d[img, ic * P:(ic + 1) * P, b0:b1])
                stgs.append(stg)
            for ic in range(2):
                stg = stgs[ic]
                ct = ct_pool.tile([P, H + 2, 4, T], BF16, tag="ct",
                                  name=f"ct_{img}_{ic}")
                for r0, r1 in ((0, 30), (30, H + 2)):
                    rs = slice(r0, r1)
                    g = nc.vector
                    g.tensor_tensor(ct[:, rs, 0, 1:], stg[:, rs, 1:54:2],
                                    stg[:, rs, 3:56:2], SUB)
                    g.tensor_tensor(ct[:, rs, 1, :], stg[:, rs, 0:56:2],
                                    stg[:, rs, 1:56:2], ADD)
                    g.tensor_tensor(ct[:, rs, 2, :], stg[:, rs, 1:56:2],
                                    stg[:, rs, 0:56:2], SUB)
                    g.tensor_tensor(ct[:, rs, 3, :27], stg[:, rs, 0:53:2],
                                    stg[:, rs, 2:55:2], SUB)
                    nc.scalar.mul(ct[:, rs, 0, 0], stg[:, rs, 1], -1.0)
                    nc.scalar.copy(ct[:, rs, 3, 27], stg[:, rs, 54])
                cts.append(ct)
            return cts

        def make_u(img, ch, cts):
            us = []
            r0 = 2 * TC * ch  # first CT row of the chunk
            for ic in range(2):
                ct = cts[ic]
                ut = u_pool.tile([P, 4, TC, 4, T], BF16, tag="u",
                                 name=f"u_{img}_{ch}_{ic}")
                a = ct[:, r0 + 0:r0 + 2 * TC - 1:2]
                b = ct[:, r0 + 1:r0 + 2 * TC:2]
                c = ct[:, r0 + 2:r0 + 2 * TC + 1:2]
                d = ct[:, r0 + 3:r0 + 2 * TC + 2:2]
                v = nc.vector
                v.tensor_tensor(ut[:, 0], a, c, SUB)
                v.tensor_tensor(ut[:, 1], b, c, ADD)
                v.tensor_tensor(ut[:, 2], c, b, SUB)
                v.tensor_tensor(ut[:, 3], b, d, SUB)
                us.append(ut)
            return us

        # ---- matmuls + output row-stage --------------------------------
        def conv_chunk(img, oc, ch, us):
            ps = [[psum_pool.tile([P, TC, T], F32, tag="ps",
                                  name=f"p_{img}_{oc}_{ch}_{u}_{j}")
                   for j in range(2)] for u in range(4)]
            for u in range(4):
                for j in range(2):
                    step = 0
                    for vs in range(3):
                        for ic in range(2):
                            v, w = wview(u, j, vs, ic, oc)
                            nc.tensor.matmul(
                                ps[u][j][:], w, us[ic][:, u, :, v, :],
                                start=(step == 0), stop=(step == 5))
                            step += 1
            # PSUM has a single DVE read port -> every op below reads at
            # most one PSUM operand. y0 = P0+P1+P2, y1 = P1-P2-P3.
            osb = osb_pool.tile([P, 2 * TC, H], F32, tag="osb",
                                name=f"osb_{img}_{oc}_{ch}")
            for j in range(2):
                c1 = tmp_pool.tile([P, TC, T], F32, tag="c1",
                                   name=f"c1_{img}_{oc}_{ch}_{j}")
                nc.scalar.copy(c1[:], ps[1][j][:])
                ta = tmp_pool.tile([P, TC, T], F32, tag="ta",
                                   name=f"ta_{img}_{oc}_{ch}_{j}")
                nc.vector.tensor_tensor(ta[:], c1[:], ps[0][j][:], ADD)
                nc.vector.tensor_tensor(osb[:, 0::2, j::2], ta[:],
                                        ps[2][j][:], ADD)
                nc.vector.tensor_tensor(ta[:], c1[:], ps[2][j][:], SUB)
                nc.vector.tensor_tensor(osb[:, 1::2, j::2], ta[:],
                                        ps[3][j][:], SUB)
            dma = nc.sync if (oc + ch) % 2 == 0 else nc.scalar
            dma.dma_start(
                y_d[img, oc * P:(oc + 1) * P, 2 * TC * ch:2 * TC * (ch + 1)],
                osb[:])

        # ---- program order ---------------------------------------------
        # CT tiles of image i are fully consumed (both make_u emitted)
        # before load_image(i+1) is emitted, so ct/stg pool recycling only
        # ever waits on already-emitted readers.
        prep_weights(0, nc.vector)
        prep_weights(1, nc.vector)
        cts = load_image(0)
        for img in range(NIMG):
            us_a = make_u(img, 0, cts)
            conv_chunk(img, 0, 0, us_a)
            us_b = make_u(img, 1, cts)
            cts = load_image(img + 1) if img + 1 < NIMG else None
            conv_chunk(img, 1, 0, us_a)
            conv_chunk(img, 0, 1, us_b)
            conv_chunk(img, 1, 1, us_b)

    nc.compile()
    return nc


def _get_program():
    if "nc" not in _cached:
        _cached["nc"] = build_program()
    return _cached["nc"]


def kernel(x: np.ndarray, W: np.ndarray, trace: bool = False, **trace_kw):
    nc = _get_program()
    x = np.ascontiguousarray(x, dtype=np.float32)
    # host-side layout permutation only (no arithmetic): [o,i,kh,kw] ->
    # [i, kh*3+kw, o]
    w_r = np.ascontiguousarray(
        np.asarray(W, dtype=np.float32).reshape(C, C, 9).transpose(1, 2, 0))
    in_maps = [{"x": x[i * NIMG:(i + 1) * NIMG], "W": w_r}
               for i in range(N_CORES)]
    res = run_bass_kernel_spmd(nc, in_maps, core_ids=list(range(N_CORES)),
                               trace=trace, **trace_kw)
    out = np.concatenate([res.results[i]["y"] for i in range(N_CORES)], axis=0)
    if trace:
        return out, res
    return out


# revision 17
# speedup vs baseline: 1.2250x; 1.2250x over previous
"""Binarized 3x3 conv (BConv) Trainium2 Bass kernel.

Problem: x[32,256,56,56] f32, W[256,256,3,3] f32.
  out = conv2d(x, sign(W), stride 1, pad 1)  (NCHW / OIHW)

Strategy:
  - Data-parallel over batch: 8 cores x 4 images each, identical SPMD program.
  - Per core: conv as 9 shifted matmuls (one per kernel tap) x 2 input-channel
    halves, accumulated in PSUM (18 steps per output tile). bf16 compute.
  - Weight prep: DMA W -> ACT cast to bf16 -> PE transpose to [C_in, C_out]
    tiles -> DVE maps w to (w>=0)-0.5 = sign(w)/2 (single tensor_scalar op;
    the x2 is folded into the PSUM eviction multiply).
  - Activations cast f32->bf16 into a zero-padded [128,58,58] SBUF image so
    no edge masking is needed.
  - Output tiles [128 out-ch, 8 rows, 56 cols] (N=448 <= one PSUM bank).
    7 row-tiles per image share one weight-load sweep (18 taps x 7 tiles).
"""

import sys
from contextlib import ExitStack

sys.path.insert(0, "/opt/trn_rl_repo")

import numpy as np

import concourse.mybir as mybir
import concourse.tile as tile
from concourse import bacc
from concourse.bass_utils import run_bass_kernel_spmd

N_CORES = 8
NIMG = 4          # images per core (32 / 8)
C = 256           # channels (in == out)
H = 56
HP = H + 2        # padded spatial
P = 128           # partitions
ROWS_PER_TILE = 8         # output rows per PSUM tile -> N = 8*56 = 448
NFT = H // ROWS_PER_TILE  # 7 row-tiles per image

F32 = mybir.dt.float32
BF16 = mybir.dt.bfloat16
FP8 = mybir.dt.float8e4
# taps computed via fp8 DoubleRow matmuls (256-deep contraction, 0.5
# cyc/col): measured e4m3 quantization of x gives 2.65% rel err on the
# conv; 4 of 9 taps in fp8 -> 2.65%*sqrt(4/9) = 1.77% < 2e-2 gate.
FP8_TAPS = (0, 1, 2, 3)
BF16_TAPS = tuple(k for k in range(9) if k not in FP8_TAPS)
# fp8 padded image block: [2, 64, 58] per partition; 64 rows (not 58) so
# the ic-half (DoubleRow pair) stride 64*58 = 3712 B is 16B-aligned.
HP8 = 64

_cached = {}


def build_program(num_devices=N_CORES):
    nc = bacc.Bacc("TRN2", target_bir_lowering=False, debug=False,
                   num_devices=num_devices)

    x_d = nc.dram_tensor("x", [NIMG, C, H, H], F32, kind="ExternalInput")
    # W arrives host-permuted to [C_in, tap, C_out] so conv matmuls can use
    # contiguous [C_in, C_out] weight slices directly (no on-chip transpose)
    w_d = nc.dram_tensor("W", [C, 9, C], F32, kind="ExternalInput")
    y_d = nc.dram_tensor("y", [NIMG, C, H, H], F32, kind="ExternalOutput")

    with tile.TileContext(nc) as tc, ExitStack() as ctx:
        wstage_pool = ctx.enter_context(tc.tile_pool(name="wstage", bufs=2))
        wbf_pool = ctx.enter_context(tc.tile_pool(name="wbf", bufs=2))
        pad_pool = ctx.enter_context(tc.tile_pool(name="pad", bufs=4))
        stage_pool = ctx.enter_context(tc.tile_pool(name="stage", bufs=3))
        out_pool = ctx.enter_context(tc.tile_pool(name="osb", bufs=6))
        psum_pool = ctx.enter_context(tc.tile_pool(name="ps", bufs=8,
                                                   space="PSUM"))

    # -- image load helper: DMA f32 chunk, cast into padded bf16 tile
    #    (for the bf16 taps) and a padded fp8 pair-tile (for the
    #    DoubleRow taps; both ic halves in one tile, dim1 = pair).
        def load_image(img, first=False):
            pads = []
            pad8 = pad_pool.tile([P, 2, HP8, HP], FP8, tag="pad8",
                                 name=f"pad8_{img}")
            for ic in range(2):
                nc.gpsimd.memset(pad8[:, ic, 0, :], 0.0)
                nc.gpsimd.memset(pad8[:, ic, HP - 1, :], 0.0)
                nc.gpsimd.memset(pad8[:, ic, 1:HP - 1, 0], 0.0)
                nc.gpsimd.memset(pad8[:, ic, 1:HP - 1, HP - 1], 0.0)
            for ic in range(2):
                pad = pad_pool.tile([P, HP, HP], BF16, tag="pad",
                                    name=f"pad_{img}_{ic}")
                # zero only the 1-px border; interior fully overwritten
                nc.gpsimd.memset(pad[:, 0, :], 0.0)
                nc.gpsimd.memset(pad[:, HP - 1, :], 0.0)
                nc.gpsimd.memset(pad[:, 1:HP - 1, 0], 0.0)
                nc.gpsimd.memset(pad[:, 1:HP - 1, HP - 1], 0.0)
                stg = stage_pool.tile([P, H, H], F32, tag="stage",
                                      name=f"stage_{img}_{ic}")
                # split DMA + cast into row halves so early row-tiles can
                # start before the whole chunk lands (subtile deps); casts
                # spread over ACT + GpSimd (1-input GpSimd copies run near
                # line rate) keeping DVE free for weight copies/evictions
                cast = (nc.scalar.copy if ic == 0
                        else nc.gpsimd.tensor_copy)
                cast8 = (nc.gpsimd.tensor_copy if ic == 0
                         else nc.scalar.copy)
                # quarter-split the very first chunk so the first conv
                # row-tiles unblock as early as possible
                n_pieces = 4 if (first and ic == 0) else 2
                step_h = H // n_pieces
                for p_i in range(n_pieces):
                    h0, h1 = p_i * step_h, (p_i + 1) * step_h
                    # first two quarters ride the ACT HWDGE ring so their
                    # completion sems race the weight DMAs on the sync ring
                    dma = (nc.scalar if (first and ic == 0 and p_i < 2)
                           else nc.sync)
                    dma.dma_start(
                        stg[:, h0:h1, :],
                        x_d[img, ic * P:(ic + 1) * P, h0:h1])
                    cast(pad[:, 1 + h0:1 + h1, 1:HP - 1], stg[:, h0:h1, :])
                    cast8(pad8[:, ic, 1 + h0:1 + h1, 1:HP - 1],
                          stg[:, h0:h1, :])
                pads.append(pad)
            return pads, pad8

        # -- weight prep: per input-channel half, one DMA + one DVE
        #    binarization ((w>=0)-0.5 = sign(w)/2, exact in bf16; the x2 is
        #    folded into the PSUM eviction). The host-permuted [i, k, o]
        #    layout means conv lhsT tiles are contiguous slices — no
        #    transposes, no copies.
        wsign = []
        w8sign = []

        def prep_weights():
            tiles = []
            w8 = wbf_pool.tile([P, 2, 9, 2 * P], FP8, tag="w8", name="w8")
            w8sign.append(w8)
            for ic in range(2):
                wst = wstage_pool.tile([P, 9, 2 * P], F32, tag="wst",
                                       name=f"wst_{ic}")
                nc.sync.dma_start(wst[:, :, :], w_d[ic * P:(ic + 1) * P])
                ws = wbf_pool.tile([P, 9, 2 * P], BF16, tag="wbf",
                                   name=f"ws_{ic}")
                tiles.append((wst, ws))
                wsign.append(ws)
            # binarize per output-channel half, oc0 halves first: the first
            # conv group gates on a 0.7us op instead of the full 1.4us sign
            # (+-0.5 is exact in both bf16 and fp8e4; the x2 is folded into
            # the PSUM eviction for both paths)
            for oc in range(2):
                for ic in range(2):
                    wst, ws = tiles[ic]
                    nc.vector.tensor_scalar(
                        ws[:, :, oc * P:(oc + 1) * P],
                        wst[:, :, oc * P:(oc + 1) * P], 0.0, 0.5,
                        mybir.AluOpType.is_ge, mybir.AluOpType.subtract)
                    nc.vector.tensor_scalar(
                        w8[:, ic, :, oc * P:(oc + 1) * P],
                        wst[:, :, oc * P:(oc + 1) * P], 0.0, 0.5,
                        mybir.AluOpType.is_ge, mybir.AluOpType.subtract)

        # -- conv for one (img, oc) group: 7 psum tiles, 18 accumulation
        #    steps each, weight-stationary inner loop over row tiles.
        def conv_group(img, oc, pads, pad8, splits=((0, NFT),),
                       cross_ring=False):
            n_steps = len(FP8_TAPS) + 2 * len(BF16_TAPS)
            for f_lo, f_hi in splits:
                psums = [psum_pool.tile([P, ROWS_PER_TILE, H], F32, tag="ps",
                                        name=f"acc_{img}_{oc}_{f}")
                         for f in range(f_lo, f_hi)]
                step = 0

                def bf16_taps(ic):
                    nonlocal step
                    for k in BF16_TAPS:
                        dh, dw = k // 3, k % 3
                        w_tile = wsign[ic][:, k, oc * P:(oc + 1) * P]
                        for i, f in enumerate(range(f_lo, f_hi)):
                            r0 = f * ROWS_PER_TILE + dh
                            nc.tensor.matmul(
                                psums[i][:],
                                w_tile[:],
                                pads[ic][:, r0:r0 + ROWS_PER_TILE,
                                         dw:dw + H],
                                start=(step == 0),
                                stop=(step == n_steps - 1),
                            )
                        step += 1

                # group starts/stops on bf16 taps; fp8 DoubleRow taps
                # (both ic halves in one 256-deep matmul) in the middle
                bf16_taps(0)
                for k in FP8_TAPS:
                    dh, dw = k // 3, k % 3
                    w_tile = w8sign[0][:, :, k, oc * P:(oc + 1) * P]
                    for i, f in enumerate(range(f_lo, f_hi)):
                        r0 = f * ROWS_PER_TILE + dh
                        nc.tensor.matmul(
                            psums[i][:],
                            w_tile,
                            pad8[:, :, r0:r0 + ROWS_PER_TILE, dw:dw + H],
                            start=(step == 0),
                            stop=(step == n_steps - 1),
                            perf_mode=mybir.MatmulPerfMode.DoubleRow,
                        )
                    step += 1
                bf16_taps(1)
                for i, f in enumerate(range(f_lo, f_hi)):
                    osb = out_pool.tile([P, ROWS_PER_TILE, H], F32,
                                        tag="osb", name=f"osb_{img}_{oc}_{f}")
                    # x2 undoes the half-scale weights; alternate evac
                    # engines so PSUM banks free up twice as fast
                    if f % 2 == 0:
                        nc.vector.tensor_scalar_mul(osb[:], psums[i][:], 2.0)
                        dma_eng = nc.scalar if cross_ring else nc.sync
                    else:
                        nc.scalar.mul(osb[:], psums[i][:], 2.0)
                        dma_eng = nc.sync
                    dma_eng.dma_start(
                        y_d[img, oc * P:(oc + 1) * P,
                            f * ROWS_PER_TILE:(f + 1) * ROWS_PER_TILE, :],
                        osb[:],
                    )

        # -- program order tuned for startup latency: W DMA first (small,
        #    gates the DVE binarization), img0 next.
        prep_weights()
        p0, q0 = load_image(0, first=True)
        conv_group(0, 0, p0, q0)
        p1, q1 = load_image(1)
        conv_group(0, 1, p0, q0)
        p2, q2 = load_image(2)
        conv_group(1, 0, p1, q1)
        conv_group(1, 1, p1, q1)
        p3, q3 = load_image(3)
        conv_group(2, 0, p2, q2)
        conv_group(2, 1, p2, q2)
        conv_group(3, 0, p3, q3)
        # final group split 4+2+1 with DMAs spread over both HWDGE rings:
        # earlier banks evacuate and DMA out while the last row-tile still
        # accumulates, shortening the kernel tail
        conv_group(3, 1, p3, q3, splits=((0, 4), (4, 6), (6, NFT)),
                   cross_ring=True)

    nc.compile()
    return nc


def _get_program():
    if "nc" not in _cached:
        _cached["nc"] = build_program()
    return _cached["nc"]


def kernel(x: np.ndarray, W: np.ndarray, trace: bool = False, **trace_kw):
    nc = _get_program()
    x = np.ascontiguousarray(x, dtype=np.float32)
    # host-side layout permutation only (no arithmetic): [o,i,kh,kw] ->
    # [i, kh*kw, o] so weight tiles are contiguous lhsT slices on device
    w_r = np.ascontiguousarray(
        np.asarray(W, dtype=np.float32).reshape(C, C, 9).transpose(1, 2, 0))
    in_maps = [{"x": x[i * NIMG:(i + 1) * NIMG], "W": w_r}
               for i in range(N_CORES)]
    res = run_bass_kernel_spmd(nc, in_maps, core_ids=list(range(N_CORES)),
                               trace=trace, **trace_kw)
    out = np.concatenate([res.results[i]["y"] for i in range(N_CORES)], axis=0)
    if trace:
        return out, res
    return out



# revision 18
# speedup vs baseline: 1.2757x; 1.0414x over previous
"""Binarized 3x3 conv (BConv) Trainium2 Bass kernel.

Problem: x[32,256,56,56] f32, W[256,256,3,3] f32.
  out = conv2d(x, sign(W), stride 1, pad 1)  (NCHW / OIHW)

Strategy:
  - Data-parallel over batch: 8 cores x 4 images each, identical SPMD program.
  - Per core: conv as 9 shifted matmuls (one per kernel tap) x 2 input-channel
    halves, accumulated in PSUM (18 steps per output tile). bf16 compute.
  - Weight prep: DMA W -> ACT cast to bf16 -> PE transpose to [C_in, C_out]
    tiles -> DVE maps w to (w>=0)-0.5 = sign(w)/2 (single tensor_scalar op;
    the x2 is folded into the PSUM eviction multiply).
  - Activations cast f32->bf16 into a zero-padded [128,58,58] SBUF image so
    no edge masking is needed.
  - Output tiles [128 out-ch, 8 rows, 56 cols] (N=448 <= one PSUM bank).
    7 row-tiles per image share one weight-load sweep (18 taps x 7 tiles).
"""

import sys
from contextlib import ExitStack

sys.path.insert(0, "/opt/trn_rl_repo")

import numpy as np

import concourse.mybir as mybir
import concourse.tile as tile
from concourse import bacc
from concourse.bass_utils import run_bass_kernel_spmd

N_CORES = 8
NIMG = 4          # images per core (32 / 8)
C = 256           # channels (in == out)
H = 56
HP = H + 2        # padded spatial
P = 128           # partitions
ROWS_PER_TILE = 8         # output rows per PSUM tile -> N = 8*56 = 448
NFT = H // ROWS_PER_TILE  # 7 row-tiles per image

F32 = mybir.dt.float32
BF16 = mybir.dt.bfloat16
FP8 = mybir.dt.float8e4
# taps computed via fp8 DoubleRow matmuls (256-deep contraction, 0.5
# cyc/col): measured e4m3 quantization of x gives 2.65% rel err on the
# conv; 4 of 9 taps in fp8 -> 2.65%*sqrt(4/9) = 1.77% < 2e-2 gate.
FP8_TAPS = (0, 1, 2, 3)
BF16_TAPS = tuple(k for k in range(9) if k not in FP8_TAPS)
# fp8 padded image block: [2, 64, 58] per partition; 64 rows (not 58) so
# the ic-half (DoubleRow pair) stride 64*58 = 3712 B is 16B-aligned.
HP8 = 64

_cached = {}


def build_program(num_devices=N_CORES):
    nc = bacc.Bacc("TRN2", target_bir_lowering=False, debug=False,
                   num_devices=num_devices)

    x_d = nc.dram_tensor("x", [NIMG, C, H, H], F32, kind="ExternalInput")
    # W arrives host-permuted to [C_in, tap, C_out] so conv matmuls can use
    # contiguous [C_in, C_out] weight slices directly (no on-chip transpose)
    w_d = nc.dram_tensor("W", [C, 9, C], F32, kind="ExternalInput")
    y_d = nc.dram_tensor("y", [NIMG, C, H, H], F32, kind="ExternalOutput")

    with tile.TileContext(nc) as tc, ExitStack() as ctx:
        wstage_pool = ctx.enter_context(tc.tile_pool(name="wstage", bufs=2))
        wbf_pool = ctx.enter_context(tc.tile_pool(name="wbf", bufs=2))
        pad_pool = ctx.enter_context(tc.tile_pool(name="pad", bufs=4))
        stage_pool = ctx.enter_context(tc.tile_pool(name="stage", bufs=3))
        out_pool = ctx.enter_context(tc.tile_pool(name="osb", bufs=6))
        psum_pool = ctx.enter_context(tc.tile_pool(name="ps", bufs=8,
                                                   space="PSUM"))

    # -- image load helper: DMA f32 chunk, cast into padded bf16 tile
    #    (for the bf16 taps) and a padded fp8 pair-tile (for the
    #    DoubleRow taps; both ic halves in one tile, dim1 = pair).
        def load_image(img, first=False):
            pads = []
            pad8 = pad_pool.tile([P, 2, HP8, HP], FP8, tag="pad8",
                                 name=f"pad8_{img}")
            for ic in range(2):
                nc.gpsimd.memset(pad8[:, ic, 0, :], 0.0)
                nc.gpsimd.memset(pad8[:, ic, HP - 1, :], 0.0)
                nc.gpsimd.memset(pad8[:, ic, 1:HP - 1, 0], 0.0)
                nc.gpsimd.memset(pad8[:, ic, 1:HP - 1, HP - 1], 0.0)
            for ic in range(2):
                pad = pad_pool.tile([P, HP, HP], BF16, tag="pad",
                                    name=f"pad_{img}_{ic}")
                # zero only the 1-px border; interior fully overwritten
                nc.gpsimd.memset(pad[:, 0, :], 0.0)
                nc.gpsimd.memset(pad[:, HP - 1, :], 0.0)
                nc.gpsimd.memset(pad[:, 1:HP - 1, 0], 0.0)
                nc.gpsimd.memset(pad[:, 1:HP - 1, HP - 1], 0.0)
                stg = stage_pool.tile([P, H, H], F32, tag="stage",
                                      name=f"stage_{img}_{ic}")
                # split DMA + cast into row halves so early row-tiles can
                # start before the whole chunk lands (subtile deps); casts
                # spread over ACT + DVE only — GpSimd CAST measures
                # ~3.1 ns/elem (unusable); DVE has headroom
                cast = (nc.scalar.copy if ic == 0
                        else nc.vector.tensor_copy)
                cast8 = (nc.vector.tensor_copy if ic == 0
                         else nc.scalar.copy)
                # quarter-split the very first chunk so the first conv
                # row-tiles unblock as early as possible
                n_pieces = 4 if (first and ic == 0) else 2
                step_h = H // n_pieces
                for p_i in range(n_pieces):
                    h0, h1 = p_i * step_h, (p_i + 1) * step_h
                    # first two quarters ride the ACT HWDGE ring so their
                    # completion sems race the weight DMAs on the sync ring
                    dma = (nc.scalar if (first and ic == 0 and p_i < 2)
                           else nc.sync)
                    dma.dma_start(
                        stg[:, h0:h1, :],
                        x_d[img, ic * P:(ic + 1) * P, h0:h1])
                    cast(pad[:, 1 + h0:1 + h1, 1:HP - 1], stg[:, h0:h1, :])
                    cast8(pad8[:, ic, 1 + h0:1 + h1, 1:HP - 1],
                          stg[:, h0:h1, :])
                pads.append(pad)
            return pads, pad8

        # -- weight prep: per input-channel half, one DMA + one DVE
        #    binarization ((w>=0)-0.5 = sign(w)/2, exact in bf16; the x2 is
        #    folded into the PSUM eviction). The host-permuted [i, k, o]
        #    layout means conv lhsT tiles are contiguous slices — no
        #    transposes, no copies.
        wsign = []
        w8sign = []

        def prep_weights():
            tiles = []
            w8 = wbf_pool.tile([P, 2, 9, 2 * P], FP8, tag="w8", name="w8")
            w8sign.append(w8)
            for ic in range(2):
                wst = wstage_pool.tile([P, 9, 2 * P], F32, tag="wst",
                                       name=f"wst_{ic}")
                nc.sync.dma_start(wst[:, :, :], w_d[ic * P:(ic + 1) * P])
                ws = wbf_pool.tile([P, 9, 2 * P], BF16, tag="wbf",
                                   name=f"ws_{ic}")
                tiles.append((wst, ws))
                wsign.append(ws)
            # binarize per output-channel half, oc0 halves first: the first
            # conv group gates on a 0.7us op instead of the full 1.4us sign
            # (+-0.5 is exact in both bf16 and fp8e4; the x2 is folded into
            # the PSUM eviction for both paths)
            for oc in range(2):
                for ic in range(2):
                    wst, ws = tiles[ic]
                    nc.vector.tensor_scalar(
                        ws[:, :, oc * P:(oc + 1) * P],
                        wst[:, :, oc * P:(oc + 1) * P], 0.0, 0.5,
                        mybir.AluOpType.is_ge, mybir.AluOpType.subtract)
                    nc.vector.tensor_scalar(
                        w8[:, ic, :, oc * P:(oc + 1) * P],
                        wst[:, :, oc * P:(oc + 1) * P], 0.0, 0.5,
                        mybir.AluOpType.is_ge, mybir.AluOpType.subtract)

        # -- conv for one (img, oc) group: 7 psum tiles, 18 accumulation
        #    steps each, weight-stationary inner loop over row tiles.
        def conv_group(img, oc, pads, pad8, splits=((0, NFT),),
                       cross_ring=False):
            n_steps = len(FP8_TAPS) + 2 * len(BF16_TAPS)
            for f_lo, f_hi in splits:
                psums = [psum_pool.tile([P, ROWS_PER_TILE, H], F32, tag="ps",
                                        name=f"acc_{img}_{oc}_{f}")
                         for f in range(f_lo, f_hi)]
                step = 0

                def bf16_taps(ic):
                    nonlocal step
                    for k in BF16_TAPS:
                        dh, dw = k // 3, k % 3
                        w_tile = wsign[ic][:, k, oc * P:(oc + 1) * P]
                        for i, f in enumerate(range(f_lo, f_hi)):
                            r0 = f * ROWS_PER_TILE + dh
                            nc.tensor.matmul(
                                psums[i][:],
                                w_tile[:],
                                pads[ic][:, r0:r0 + ROWS_PER_TILE,
                                         dw:dw + H],
                                start=(step == 0),
                                stop=(step == n_steps - 1),
                            )
                        step += 1

                # group starts/stops on bf16 taps; fp8 DoubleRow taps
                # (both ic halves in one 256-deep matmul) in the middle
                bf16_taps(0)
                for k in FP8_TAPS:
                    dh, dw = k // 3, k % 3
                    w_tile = w8sign[0][:, :, k, oc * P:(oc + 1) * P]
                    for i, f in enumerate(range(f_lo, f_hi)):
                        r0 = f * ROWS_PER_TILE + dh
                        nc.tensor.matmul(
                            psums[i][:],
                            w_tile,
                            pad8[:, :, r0:r0 + ROWS_PER_TILE, dw:dw + H],
                            start=(step == 0),
                            stop=(step == n_steps - 1),
                            perf_mode=mybir.MatmulPerfMode.DoubleRow,
                        )
                    step += 1
                bf16_taps(1)
                for i, f in enumerate(range(f_lo, f_hi)):
                    osb = out_pool.tile([P, ROWS_PER_TILE, H], F32,
                                        tag="osb", name=f"osb_{img}_{oc}_{f}")
                    # x2 undoes the half-scale weights; alternate evac
                    # engines so PSUM banks free up twice as fast
                    if f % 2 == 0:
                        nc.vector.tensor_scalar_mul(osb[:], psums[i][:], 2.0)
                        dma_eng = nc.scalar if cross_ring else nc.sync
                    else:
                        nc.scalar.mul(osb[:], psums[i][:], 2.0)
                        dma_eng = nc.sync
                    dma_eng.dma_start(
                        y_d[img, oc * P:(oc + 1) * P,
                            f * ROWS_PER_TILE:(f + 1) * ROWS_PER_TILE, :],
                        osb[:],
                    )

        # -- program order tuned for startup latency: W DMA first (small,
        #    gates the DVE binarization), img0 next.
        prep_weights()
        p0, q0 = load_image(0, first=True)
        conv_group(0, 0, p0, q0)
        p1, q1 = load_image(1)
        conv_group(0, 1, p0, q0)
        p2, q2 = load_image(2)
        conv_group(1, 0, p1, q1)
        conv_group(1, 1, p1, q1)
        p3, q3 = load_image(3)
        conv_group(2, 0, p2, q2)
        conv_group(2, 1, p2, q2)
        conv_group(3, 0, p3, q3)
        # final group split 4+2+1 with DMAs spread over both HWDGE rings:
        # earlier banks evacuate and DMA out while the last row-tile still
        # accumulates, shortening the kernel tail
        conv_group(3, 1, p3, q3, splits=((0, 4), (4, 6), (6, NFT)),
                   cross_ring=True)

    nc.compile()
    return nc


def _get_program():
    if "nc" not in _cached:
        _cached["nc"] = build_program()
    return _cached["nc"]


def kernel(x: np.ndarray, W: np.ndarray, trace: bool = False, **trace_kw):
    nc = _get_program()
    x = np.ascontiguousarray(x, dtype=np.float32)
    # host-side layout permutation only (no arithmetic): [o,i,kh,kw] ->
    # [i, kh*kw, o] so weight tiles are contiguous lhsT slices on device
    w_r = np.ascontiguousarray(
        np.asarray(W, dtype=np.float32).reshape(C, C, 9).transpose(1, 2, 0))
    in_maps = [{"x": x[i * NIMG:(i + 1) * NIMG], "W": w_r}
               for i in range(N_CORES)]
    res = run_bass_kernel_spmd(nc, in_maps, core_ids=list(range(N_CORES)),
                               trace=trace, **trace_kw)
    out = np.concatenate([res.results[i]["y"] for i in range(N_CORES)], axis=0)
    if trace:
        return out, res
    return out

